# revision 2
# baseline (speedup 1.0000x reference)
"""Augmented Neural ODE kernel for 8 TRN2 NeuronCores.

Data-parallel over the batch dim (8 batches/core -> 512 tokens/core).
State kept feature-major [STATE=128 partitions, 512 tokens] in SBUF; the
whole MLP (128->1024->1024->1024->128) runs per Euler step as 145
accumulating f32r matmuls + tanh on the scalar engine. The Euler update
y' = y + dt*f is folded into the last PSUM accumulation group via an
identity-weight matmul (W3 pre-scaled by dt on the host).

All matmul operands are float32r (4-byte storage, reduced-precision PE
mode, 1 cycle/row at N=512 = 4x the fp32 matmul rate); every producer
writing a matmul operand outputs f32r as the BIR verifier requires.
"""

import os

os.environ.setdefault("JAX_PLATFORMS", "cpu")

import sys

if "/opt/trn_rl_repo" not in sys.path:
    sys.path.insert(0, "/opt/trn_rl_repo")

import numpy as np

B, S, DIN, DAUG = 64, 64, 64, 64
STATE = DIN + DAUG          # 128
HID = 1024
T = 32
NCORES = 8
BSHARD = B // NCORES        # 8
NTOK = BSHARD * S           # 512 tokens per core
KC = HID // 128             # 8 chunks of the hidden dim

_cached_nc = None


def _build():
    global _cached_nc
    if _cached_nc is not None:
        return _cached_nc

    import concourse.tile as tile
    from concourse import bacc, mybir

    f32 = mybir.dt.float32
    f32r = mybir.dt.float32r
    Tanh = mybir.ActivationFunctionType.Tanh
    Ident = mybir.ActivationFunctionType.Identity

    nc = bacc.Bacc("TRN2", target_bir_lowering=False, debug=False,
                   num_devices=NCORES)

    y0t_d = nc.dram_tensor("y0t", [DIN, NTOK], f32r, kind="ExternalInput").ap()
    laug_d = nc.dram_tensor("laug", [DIN, STATE], f32r, kind="ExternalInput").ap()
    baug_d = nc.dram_tensor("baug", [STATE, 1], f32, kind="ExternalInput").ap()
    w0t_d = nc.dram_tensor("w0t", [STATE, HID], f32r, kind="ExternalInput").ap()
    w1t_d = nc.dram_tensor("w1t", [KC, 128, HID], f32r, kind="ExternalInput").ap()
    w2t_d = nc.dram_tensor("w2t", [KC, 128, HID], f32r, kind="ExternalInput").ap()
    w3t_d = nc.dram_tensor("w3t", [KC, 128, STATE], f32r, kind="ExternalInput").ap()
    b0_d = nc.dram_tensor("b0", [128, KC], f32, kind="ExternalInput").ap()
    b1_d = nc.dram_tensor("b1", [128, KC], f32, kind="ExternalInput").ap()
    b2_d = nc.dram_tensor("b2", [128, KC], f32, kind="ExternalInput").ap()
    b3dt_d = nc.dram_tensor("b3dt", [STATE, 1], f32, kind="ExternalInput").ap()
    idt_d = nc.dram_tensor("idt", [STATE, STATE], f32r, kind="ExternalInput").ap()
    out_d = nc.dram_tensor("out", [DIN, NTOK], f32r, kind="ExternalOutput").ap()

    with tile.TileContext(nc) as tc:
        with tc.tile_pool(name="wpool", bufs=1) as wpool, \
             tc.tile_pool(name="hpool", bufs=24) as hpool, \
             tc.tile_pool(name="ypool", bufs=2) as ypool, \
             tc.tile_pool(name="pspool", bufs=4, space="PSUM") as pspool:

            w0t = wpool.tile([128, HID], f32r)
            nc.sync.dma_start(w0t[:], w0t_d[:])
            w1t = wpool.tile([128, KC, HID], f32r)
            w2t = wpool.tile([128, KC, HID], f32r)
            w3t = wpool.tile([128, KC, STATE], f32r)
            for g in range(KC):
                nc.sync.dma_start(w1t[:, g, :], w1t_d[g])
            for g in range(KC):
                nc.sync.dma_start(w2t[:, g, :], w2t_d[g])
            for g in range(KC):
                nc.sync.dma_start(w3t[:, g, :], w3t_d[g])
            idt = wpool.tile([128, STATE], f32r)
            nc.sync.dma_start(idt[:], idt_d[:])
            b0 = wpool.tile([128, KC], f32)
            nc.sync.dma_start(b0[:], b0_d[:])
            b1 = wpool.tile([128, KC], f32)
            nc.sync.dma_start(b1[:], b1_d[:])
            b2 = wpool.tile([128, KC], f32)
            nc.sync.dma_start(b2[:], b2_d[:])
            baug = wpool.tile([128, 1], f32)
            nc.sync.dma_start(baug[:], baug_d[:])
            b3dt = wpool.tile([128, 1], f32)
            nc.sync.dma_start(b3dt[:], b3dt_d[:])

            laug = wpool.tile([DIN, STATE], f32r)
            nc.sync.dma_start(laug[:], laug_d[:])
            y0t = wpool.tile([DIN, NTOK], f32r)
            nc.sync.dma_start(y0t[:], y0t_d[:])

            # augment: y = [y0; W_aug y0 + b_aug]   (K = 64, one-time)
            ps = pspool.tile([128, NTOK], f32, tag="ps")
            nc.tensor.matmul(ps[:], lhsT=laug[:], rhs=y0t[:],
                             start=True, stop=True)
            y = ypool.tile([128, NTOK], f32r, tag="y")
            nc.scalar.activation(y[:], ps[:], Ident, bias=baug[:, 0:1])

            for _step in range(T - 1):
                h0 = []
                for m in range(KC):
                    ps = pspool.tile([128, NTOK], f32, tag="ps")
                    nc.tensor.matmul(ps[:], lhsT=w0t[:, m * 128:(m + 1) * 128],
                                     rhs=y[:], start=True, stop=True)
                    h = hpool.tile([128, NTOK], f32r, tag="h")
                    nc.scalar.activation(h[:], ps[:], Tanh, bias=b0[:, m:m + 1])
                    h0.append(h)
                h1 = []
                for m in range(KC):
                    ps = pspool.tile([128, NTOK], f32, tag="ps")
                    for g in range(KC):
                        nc.tensor.matmul(ps[:],
                                         lhsT=w1t[:, g, m * 128:(m + 1) * 128],
                                         rhs=h0[g][:],
                                         start=(g == 0), stop=(g == KC - 1))
                    h = hpool.tile([128, NTOK], f32r, tag="h")
                    nc.scalar.activation(h[:], ps[:], Tanh, bias=b1[:, m:m + 1])
                    h1.append(h)
                h2 = []
                for m in range(KC):
                    ps = pspool.tile([128, NTOK], f32, tag="ps")
                    for g in range(KC):
                        nc.tensor.matmul(ps[:],
                                         lhsT=w2t[:, g, m * 128:(m + 1) * 128],
                                         rhs=h1[g][:],
                                         start=(g == 0), stop=(g == KC - 1))
                    h = hpool.tile([128, NTOK], f32r, tag="h")
                    nc.scalar.activation(h[:], ps[:], Tanh, bias=b2[:, m:m + 1])
                    h2.append(h)
                # layer 3 (pre-scaled by dt on host) + Euler carry via I-matmul
                ps = pspool.tile([128, NTOK], f32, tag="ps")
                for g in range(KC):
                    nc.tensor.matmul(ps[:], lhsT=w3t[:, g, :], rhs=h2[g][:],
                                     start=(g == 0), stop=False)
                nc.tensor.matmul(ps[:], lhsT=idt[:], rhs=y[:],
                                 start=False, stop=True)
                y_next = ypool.tile([128, NTOK], f32r, tag="y")
                nc.scalar.activation(y_next[:], ps[:], Ident, bias=b3dt[:, 0:1])
                y = y_next

            nc.sync.dma_start(out_d[:], y[0:DIN, :])

    nc.compile()
    _cached_nc = nc
    return nc


def _make_in_maps(y0, t, W_aug, b_aug, W0, b0, W1, b1, W2, b2, W3, b3):
    f = np.float32
    dt = float(np.asarray(t, dtype=f)[1] - np.asarray(t, dtype=f)[0])
    laug = np.concatenate([np.eye(DIN, dtype=f),
                           np.asarray(W_aug, f).T], axis=1)          # [64, 128]
    baug = np.concatenate([np.zeros(DIN, f),
                           np.asarray(b_aug, f)]).reshape(STATE, 1)
    w0t = np.ascontiguousarray(np.asarray(W0, f).T)                  # [128,1024]
    w1t = np.ascontiguousarray(np.asarray(W1, f).T.reshape(KC, 128, HID))
    w2t = np.ascontiguousarray(np.asarray(W2, f).T.reshape(KC, 128, HID))
    w3t = np.ascontiguousarray((dt * np.asarray(W3, f)).T.reshape(KC, 128, STATE))
    b0r = np.ascontiguousarray(np.asarray(b0, f).reshape(KC, 128).T)
    b1r = np.ascontiguousarray(np.asarray(b1, f).reshape(KC, 128).T)
    b2r = np.ascontiguousarray(np.asarray(b2, f).reshape(KC, 128).T)
    b3dt = (dt * np.asarray(b3, f)).reshape(STATE, 1)
    idt = np.eye(STATE, dtype=f)  # carries y through the accumulation group

    shared = dict(laug=laug, baug=baug, w0t=w0t, w1t=w1t, w2t=w2t, w3t=w3t,
                  b0=b0r, b1=b1r, b2=b2r, b3dt=b3dt, idt=idt)
    in_maps = []
    for c in range(NCORES):
        y0c = np.ascontiguousarray(
            np.asarray(y0, f)[c * BSHARD:(c + 1) * BSHARD]
            .reshape(NTOK, DIN).T)                                   # [64, 512]
        in_maps.append(dict(y0t=y0c, **shared))
    return in_maps


def _run(inputs, trace=False, **trace_kwargs):
    from concourse.bass_utils import run_bass_kernel_spmd

    nc = _build()
    in_maps = _make_in_maps(**inputs)
    res = run_bass_kernel_spmd(nc, in_maps, core_ids=list(range(NCORES)),
                               trace=trace, **trace_kwargs)
    outs = [res.results[c]["out"] for c in range(NCORES)]            # [64,512]
    full = np.concatenate(
        [o.T.reshape(BSHARD, S, DIN) for o in outs], axis=0)
    return np.ascontiguousarray(full, dtype=np.float32), res


def kernel(**inputs):
    out, _ = _run(inputs, trace=False)
    return out


# revision 4
# speedup vs baseline: 1.7973x; 1.7973x over previous
"""Augmented Neural ODE kernel for 8 TRN2 NeuronCores.

Data-parallel over the batch dim (8 batches/core -> 512 tokens/core).
State kept feature-major [STATE=128 partitions, 512 tokens] in SBUF; the
whole MLP (128->1024->1024->1024->128) runs per Euler step as 145
accumulating matmuls + tanh on the scalar engine.

MLP matmuls run in bf16 (2 elems/cycle XBUS streaming + fast weight
load); the Euler carry y' = y + dt*f runs at f32r precision: W3 is
pre-scaled by dt on the host, an identity-weight f32r matmul adds y into
the same PSUM accumulation group, and the scalar engine writes the new
f32r state (bf16 carry would accumulate ~4e-2 error over 31 steps; f32r
keeps it ~1e-3). A vector-engine copy produces the bf16 view of y that
feeds the next step's layer-0 matmuls.
"""

import os

os.environ.setdefault("JAX_PLATFORMS", "cpu")

import sys

if "/opt/trn_rl_repo" not in sys.path:
    sys.path.insert(0, "/opt/trn_rl_repo")

import numpy as np

B, S, DIN, DAUG = 64, 64, 64, 64
STATE = DIN + DAUG          # 128
HID = 1024
T = 32
NCORES = 8
BSHARD = B // NCORES        # 8
NTOK = BSHARD * S           # 512 tokens per core
KC = HID // 128             # 8 chunks of the hidden dim

_cached_nc = None


def _build():
    global _cached_nc
    if _cached_nc is not None:
        return _cached_nc

    import concourse.tile as tile
    from concourse import bacc, mybir

    f32 = mybir.dt.float32
    f32r = mybir.dt.float32r
    bf16 = mybir.dt.bfloat16
    Tanh = mybir.ActivationFunctionType.Tanh
    Ident = mybir.ActivationFunctionType.Identity

    nc = bacc.Bacc("TRN2", target_bir_lowering=False, debug=False,
                   num_devices=NCORES)

    y0t_d = nc.dram_tensor("y0t", [DIN, NTOK], f32r, kind="ExternalInput").ap()
    laug_d = nc.dram_tensor("laug", [DIN, STATE], f32r, kind="ExternalInput").ap()
    baug_d = nc.dram_tensor("baug", [STATE, 1], f32, kind="ExternalInput").ap()
    w0t_d = nc.dram_tensor("w0t", [STATE, HID], bf16, kind="ExternalInput").ap()
    w1t_d = nc.dram_tensor("w1t", [KC, 128, HID], bf16, kind="ExternalInput").ap()
    w2t_d = nc.dram_tensor("w2t", [KC, 128, HID], bf16, kind="ExternalInput").ap()
    w3t_d = nc.dram_tensor("w3t", [KC, 128, STATE], bf16, kind="ExternalInput").ap()
    b0_d = nc.dram_tensor("b0", [128, KC], f32, kind="ExternalInput").ap()
    b1_d = nc.dram_tensor("b1", [128, KC], f32, kind="ExternalInput").ap()
    b2_d = nc.dram_tensor("b2", [128, KC], f32, kind="ExternalInput").ap()
    b3dt_d = nc.dram_tensor("b3dt", [STATE, 1], f32, kind="ExternalInput").ap()
    idt_d = nc.dram_tensor("idt", [STATE, STATE], f32r, kind="ExternalInput").ap()
    out_d = nc.dram_tensor("out", [DIN, NTOK], f32r, kind="ExternalOutput").ap()

    with tile.TileContext(nc) as tc:
        with tc.tile_pool(name="wpool", bufs=1) as wpool, \
             tc.tile_pool(name="hpool", bufs=24) as hpool, \
             tc.tile_pool(name="ypool", bufs=2) as ypool, \
             tc.tile_pool(name="ybpool", bufs=2) as ybpool, \
             tc.tile_pool(name="pspool", bufs=8, space="PSUM") as pspool:

            # weight loads spread across 4 DMA-hosting engines so the
            # first Euler step isn't serialized behind one queue
            w0t = wpool.tile([128, HID], bf16)
            nc.sync.dma_start(w0t[:], w0t_d[:])
            laug = wpool.tile([DIN, STATE], f32r)
            nc.sync.dma_start(laug[:], laug_d[:])
            y0t = wpool.tile([DIN, NTOK], f32r)
            nc.sync.dma_start(y0t[:], y0t_d[:])

            w1t = wpool.tile([128, KC, HID], bf16)
            w2t = wpool.tile([128, KC, HID], bf16)
            w3t = wpool.tile([128, KC, STATE], bf16)
            for g in range(KC):
                nc.gpsimd.dma_start(w1t[:, g, :], w1t_d[g])
            for g in range(KC):
                nc.scalar.dma_start(w2t[:, g, :], w2t_d[g])
            for g in range(KC):
                nc.gpsimd.dma_start(w3t[:, g, :], w3t_d[g])
            idt = wpool.tile([128, STATE], f32r)
            nc.scalar.dma_start(idt[:], idt_d[:])
            b0 = wpool.tile([128, KC], f32)
            nc.sync.dma_start(b0[:], b0_d[:])
            b1 = wpool.tile([128, KC], f32)
            nc.sync.dma_start(b1[:], b1_d[:])
            b2 = wpool.tile([128, KC], f32)
            nc.sync.dma_start(b2[:], b2_d[:])
            baug = wpool.tile([128, 1], f32)
            nc.sync.dma_start(baug[:], baug_d[:])
            b3dt = wpool.tile([128, 1], f32)
            nc.sync.dma_start(b3dt[:], b3dt_d[:])

            # augment: y = [y0; W_aug y0 + b_aug]   (K = 64, one-time)
            ps = pspool.tile([128, NTOK], f32, tag="ps")
            nc.tensor.matmul(ps[:], lhsT=laug[:], rhs=y0t[:],
                             start=True, stop=True)
            y = ypool.tile([128, NTOK], f32r, tag="y")
            nc.scalar.activation(y[:], ps[:], Ident, bias=baug[:, 0:1])
            yb = ybpool.tile([128, NTOK], bf16, tag="yb")
            nc.vector.tensor_copy(yb[:], y[:])

            for _step in range(T - 1):
                h0 = []
                for m in range(KC):
                    ps = pspool.tile([128, NTOK], f32, tag="ps")
                    nc.tensor.matmul(ps[:], lhsT=w0t[:, m * 128:(m + 1) * 128],
                                     rhs=yb[:], start=True, stop=True)
                    h = hpool.tile([128, NTOK], bf16, tag="h")
                    nc.scalar.activation(h[:], ps[:], Tanh, bias=b0[:, m:m + 1])
                    h0.append(h)
                h1 = []
                for m in range(KC):
                    ps = pspool.tile([128, NTOK], f32, tag="ps")
                    for g in range(KC):
                        nc.tensor.matmul(ps[:],
                                         lhsT=w1t[:, g, m * 128:(m + 1) * 128],
                                         rhs=h0[g][:],
                                         start=(g == 0), stop=(g == KC - 1))
                    h = hpool.tile([128, NTOK], bf16, tag="h")
                    nc.scalar.activation(h[:], ps[:], Tanh, bias=b1[:, m:m + 1])
                    h1.append(h)
                h2 = []
                for m in range(KC):
                    ps = pspool.tile([128, NTOK], f32, tag="ps")
                    for g in range(KC):
                        nc.tensor.matmul(ps[:],
                                         lhsT=w2t[:, g, m * 128:(m + 1) * 128],
                                         rhs=h1[g][:],
                                         start=(g == 0), stop=(g == KC - 1))
                    h = hpool.tile([128, NTOK], bf16, tag="h")
                    nc.scalar.activation(h[:], ps[:], Tanh, bias=b2[:, m:m + 1])
                    h2.append(h)
                # layer 3 (pre-scaled by dt on host) + Euler carry via
                # f32r identity matmul into the same accumulation group
                ps = pspool.tile([128, NTOK], f32, tag="ps")
                for g in range(KC):
                    nc.tensor.matmul(ps[:], lhsT=w3t[:, g, :], rhs=h2[g][:],
                                     start=(g == 0), stop=False)
                nc.tensor.matmul(ps[:], lhsT=idt[:], rhs=y[:],
                                 start=False, stop=True)
                y = ypool.tile([128, NTOK], f32r, tag="y")
                nc.scalar.activation(y[:], ps[:], Ident, bias=b3dt[:, 0:1])
                yb = ybpool.tile([128, NTOK], bf16, tag="yb")
                nc.vector.tensor_copy(yb[:], y[:])

            nc.sync.dma_start(out_d[:], y[0:DIN, :])

    nc.compile()
    _cached_nc = nc
    return nc


def _make_in_maps(y0, t, W_aug, b_aug, W0, b0, W1, b1, W2, b2, W3, b3):
    import ml_dtypes
    f = np.float32
    bf = ml_dtypes.bfloat16
    dt = float(np.asarray(t, dtype=f)[1] - np.asarray(t, dtype=f)[0])
    laug = np.concatenate([np.eye(DIN, dtype=f),
                           np.asarray(W_aug, f).T], axis=1)          # [64, 128]
    baug = np.concatenate([np.zeros(DIN, f),
                           np.asarray(b_aug, f)]).reshape(STATE, 1)
    w0t = np.ascontiguousarray(np.asarray(W0, f).T).astype(bf)       # [128,1024]
    w1t = np.ascontiguousarray(np.asarray(W1, f).T.reshape(KC, 128, HID)).astype(bf)
    w2t = np.ascontiguousarray(np.asarray(W2, f).T.reshape(KC, 128, HID)).astype(bf)
    w3t = np.ascontiguousarray(
        (dt * np.asarray(W3, f)).T.reshape(KC, 128, STATE)).astype(bf)
    b0r = np.ascontiguousarray(np.asarray(b0, f).reshape(KC, 128).T)
    b1r = np.ascontiguousarray(np.asarray(b1, f).reshape(KC, 128).T)
    b2r = np.ascontiguousarray(np.asarray(b2, f).reshape(KC, 128).T)
    b3dt = (dt * np.asarray(b3, f)).reshape(STATE, 1)
    idt = np.eye(STATE, dtype=f)  # carries y through the accumulation group

    shared = dict(laug=laug, baug=baug, w0t=w0t, w1t=w1t, w2t=w2t, w3t=w3t,
                  b0=b0r, b1=b1r, b2=b2r, b3dt=b3dt, idt=idt)
    in_maps = []
    for c in range(NCORES):
        y0c = np.ascontiguousarray(
            np.asarray(y0, f)[c * BSHARD:(c + 1) * BSHARD]
            .reshape(NTOK, DIN).T)                                   # [64, 512]
        in_maps.append(dict(y0t=y0c, **shared))
    return in_maps


def _run(inputs, trace=False, **trace_kwargs):
    from concourse.bass_utils import run_bass_kernel_spmd

    nc = _build()
    in_maps = _make_in_maps(**inputs)
    res = run_bass_kernel_spmd(nc, in_maps, core_ids=list(range(NCORES)),
                               trace=trace, **trace_kwargs)
    outs = [res.results[c]["out"] for c in range(NCORES)]            # [64,512]
    full = np.concatenate(
        [o.T.reshape(BSHARD, S, DIN) for o in outs], axis=0)
    return np.ascontiguousarray(full, dtype=np.float32), res


def kernel(**inputs):
    out, _ = _run(inputs, trace=False)
    return out
